# revision 24
# baseline (speedup 1.0000x reference)
"""Lucas-Kanade delta_p kernel for 8 trn2 NeuronCores.

Strategy (dense per-pixel product maps, no on-device gather):
Every per-point output derives from 15x15 box-sums of five per-pixel
product maps (Ix^2, IxIy, Iy^2, Ix*E, Iy*E with E = img2-img1).  Points
lie in [0,1000)^2 so only the top-left ~1016x1016 corner matters.  Each
core owns a 125-row y-band (139 sobel rows incl. halo) and computes,
densely for all x:
 - full Sobel (vertical taps via banded lhsT, horizontal taps via
   shifted rhs views) as accumulating bf16 matmuls on the PE, split
   into a 116-row main tier and a 32-row bottom tier so no contraction
   exceeds 128 partitions
 - the five per-pixel product maps on ACT (squares) / DVE / GpSimd,
   written as bf16 directly into the output staging tile
The host finishes with a float64 2D integral image per map (the 15x15
box-sum) and the closed-form 2x2 solve at the 100k point locations.
No cross-core communication, no gather.
"""

import numpy as np
import ml_dtypes

import concourse.bass as bass
import concourse.bacc as bacc
import concourse.mybir as mybir
from concourse.tile import TileContext
from concourse.bass_utils import run_bass_kernel_spmd

F32 = mybir.dt.float32
BF16 = mybir.dt.bfloat16

NCORES = 8
BAND = 125          # output band rows per core
TA = 116            # main-tier image rows (sobel rows 0..113)
TB = 32             # bottom-tier image rows (img rows 114..145)
RA = 114            # valid sobel rows in tier A
RB = 25             # valid sobel rows in tier B (114..138)
IMG_ROWS = 146
CLD = 1040          # image columns loaded (shifted reads up to 1026)
CW = 1024           # working column width
XP = 1016           # product-map x columns that matter
PATCH = 15

AL = mybir.AluOpType
AF = mybir.ActivationFunctionType

# block offsets inside the packed weight tiles
_WA = {"smA": 0, "smAn": 128, "dfA": 256, "dfA2": 384}
_WB = {"smB": 0, "smBn": 32, "dfB": 64, "dfB2": 96}


def _packed_weights():
    sm = (2.0, 4.0, 2.0)
    df = (2.0, 0.0, -2.0)
    smA = np.zeros((128, 128), np.float32)   # sobel rows 0..113 from tier A
    dfA = np.zeros((128, 128), np.float32)
    for m in range(RA):
        for u in range(3):
            smA[m + u, m] = sm[u]
            dfA[m + u, m] = df[u]
    smB = np.zeros((32, 32), np.float32)     # sobel rows 114..138 from tier B
    dfB = np.zeros((32, 32), np.float32)
    for mB in range(RB):
        for u in range(3):
            smB[mB + u, mB] = sm[u]
            dfB[mB + u, mB] = df[u]
    wp = np.zeros((128, 640), np.float32)
    for nm, blk in (("smA", smA), ("smAn", -smA), ("dfA", dfA),
                    ("dfA2", 2.0 * dfA)):
        wp[:, _WA[nm]:_WA[nm] + 128] = blk
    for nm, blk in (("smB", smB), ("smBn", -smB), ("dfB", dfB),
                    ("dfB2", 2.0 * dfB)):
        wp[0:32, 512 + _WB[nm]:512 + _WB[nm] + 32] = blk
    return wp.astype(ml_dtypes.bfloat16)


def build_core_inputs(img1, img2):
    im1 = np.asarray(img1).reshape(img1.shape[-2], img1.shape[-1])
    im2 = np.asarray(img2).reshape(img2.shape[-2], img2.shape[-1])
    wp = _packed_weights()
    in_maps = []
    for c in range(NCORES):
        r0 = c * BAND
        in_maps.append(dict(
            img1b=np.ascontiguousarray(
                im1[r0:r0 + IMG_ROWS, :CLD].astype(ml_dtypes.bfloat16)),
            img2b=np.ascontiguousarray(
                im2[r0:r0 + IMG_ROWS, :CLD].astype(ml_dtypes.bfloat16)),
            wp=wp))
    return in_maps


_prog_cache = {}


def build_program():
    if "p" in _prog_cache:
        return _prog_cache["p"]
    nc = bacc.Bacc(None, target_bir_lowering=False, debug=True)
    img1b = nc.declare_dram_parameter("img1b", [IMG_ROWS, CLD], BF16, isOutput=False)
    img2b = nc.declare_dram_parameter("img2b", [IMG_ROWS, CLD], BF16, isOutput=False)
    wp_d = nc.declare_dram_parameter("wp", [128, 640], BF16, isOutput=False)
    # per-partition free layout: [chunk(2), map(5), 512]
    outA = nc.declare_dram_parameter("outA", [RA, 5120], BF16, isOutput=True)
    outB = nc.declare_dram_parameter("outB", [RB, 5120], BF16, isOutput=True)

    with TileContext(nc) as tc:
        with tc.tile_pool(name="cn", bufs=1) as cn, \
             tc.tile_pool(name="ps", bufs=8, space="PSUM") as ps:
            # ---- loads: everything bf16, sobel-critical first -----------
            i1A = cn.tile([TA, CLD], BF16, tag="i1A")
            i1B = cn.tile([TB, CLD], BF16, tag="i1B")
            i2A = cn.tile([TA, CLD], BF16, tag="i2A")
            i2B = cn.tile([TB, CLD], BF16, tag="i2B")
            wp = cn.tile([128, 640], BF16, tag="wp")
            # SP and ACT have hardware DGE queues; GpSimd's is software
            # (slow) so it never issues DMAs.  i1A/i1B/wp gate the matmuls.
            nc.sync.dma_start(out=i1A[:], in_=img1b[0:TA, :])
            nc.scalar.dma_start(out=wp[:], in_=wp_d[:])
            nc.sync.dma_start(out=i1B[:], in_=img1b[114:146, :])
            nc.scalar.dma_start(out=i2A[:], in_=img2b[0:TA, :])
            nc.sync.dma_start(out=i2B[:], in_=img2b[114:146, :])

            def WA(name):
                return wp[0:TA, _WA[name]:_WA[name] + TA]

            def WB(name):
                return wp[0:32, 512 + _WB[name]:512 + _WB[name] + 32]

            # ---- persistent SBUF tiles ---------------------------------
            IyAs = cn.tile([TA, CW], F32, tag="IyAs")
            IyBs = cn.tile([TB, CW], F32, tag="IyBs")
            EA = cn.tile([TA, CW], F32, tag="EA")
            EB = cn.tile([TB, CW], F32, tag="EB")
            otA = cn.tile([TA, 5120], BF16, tag="otA")
            otB = cn.tile([TB, 5120], BF16, tag="otB")

            for ic, c0 in enumerate((0, 512)):
                def shA(s):
                    return i1A[:, c0 + s:c0 + s + 512]

                def shB(s):
                    return i1B[:, c0 + s:c0 + s + 512]
                o = slice(c0, c0 + 512)
                # Sobel: Ix = vsm[c] - vsm[c+2]; Iy = vdf[c]+2vdf[c+1]+vdf[c+2]
                IxA = ps.tile([TA, 512], F32, tag="bank", name=f"IxA{ic}")
                nc.tensor.matmul(out=IxA[:], lhsT=WA("smA"), rhs=shA(0),
                                 start=True, stop=False)
                nc.tensor.matmul(out=IxA[:], lhsT=WA("smAn"), rhs=shA(2),
                                 start=False, stop=True)
                IxB = ps.tile([TB, 512], F32, tag="bank", name=f"IxB{ic}")
                nc.tensor.matmul(out=IxB[:], lhsT=WB("smB"), rhs=shB(0),
                                 start=True, stop=False)
                nc.tensor.matmul(out=IxB[:], lhsT=WB("smBn"), rhs=shB(2),
                                 start=False, stop=True)
                IyA = ps.tile([TA, 512], F32, tag="bank", name=f"IyA{ic}")
                nc.tensor.matmul(out=IyA[:], lhsT=WA("dfA"), rhs=shA(0),
                                 start=True, stop=False)
                nc.tensor.matmul(out=IyA[:], lhsT=WA("dfA2"), rhs=shA(1),
                                 start=False, stop=False)
                nc.tensor.matmul(out=IyA[:], lhsT=WA("dfA"), rhs=shA(2),
                                 start=False, stop=True)
                IyB = ps.tile([TB, 512], F32, tag="bank", name=f"IyB{ic}")
                nc.tensor.matmul(out=IyB[:], lhsT=WB("dfB"), rhs=shB(0),
                                 start=True, stop=False)
                nc.tensor.matmul(out=IyB[:], lhsT=WB("dfB2"), rhs=shB(1),
                                 start=False, stop=False)
                nc.tensor.matmul(out=IyB[:], lhsT=WB("dfB"), rhs=shB(2),
                                 start=False, stop=True)

                # E and Iy-to-SBUF
                nc.vector.tensor_tensor(out=EA[:, o], in0=i2A[:, o],
                                        in1=i1A[:, o], op=AL.subtract)
                nc.vector.tensor_tensor(out=EB[:, o], in0=i2B[:, o],
                                        in1=i1B[:, o], op=AL.subtract)
                nc.scalar.copy(out=IyAs[:, o], in_=IyA[:])
                nc.scalar.copy(out=IyBs[:, o], in_=IyB[:])

                # products straight into the bf16 staging tiles
                def dst(ott, ci):
                    base = ic * 2560 + ci * 512
                    return ott[:, base:base + 512]

                for tier, Ixp, Iys, Ep, ott in (
                        ("A", IxA, IyAs, EA, otA), ("B", IxB, IyBs, EB, otB)):
                    nc.scalar.activation(out=dst(ott, 0), in_=Ixp[:],
                                         func=AF.Square)
                    nc.scalar.activation(out=dst(ott, 2), in_=Iys[:, o],
                                         func=AF.Square)
                    nc.vector.tensor_tensor(out=dst(ott, 1), in0=Ixp[:],
                                            in1=Iys[:, o], op=AL.mult)
                    nc.vector.tensor_tensor(out=dst(ott, 3), in0=Ixp[:],
                                            in1=Ep[:, o], op=AL.mult)
                    eng4 = nc.gpsimd if tier == "A" else nc.vector
                    eng4.tensor_tensor(out=dst(ott, 4), in0=Iys[:, o],
                                       in1=Ep[:, o], op=AL.mult)

                if ic == 1:
                    # one row-set over both chunks: half the DMA packets
                    nc.sync.dma_start(out=outA[0:48, :], in_=otA[0:48, :])
                    nc.scalar.dma_start(out=outA[48:96, :],
                                        in_=otA[48:96, :])
                    nc.sync.dma_start(out=outA[96:RA, :],
                                      in_=otA[96:RA, :])
                    nc.gpsimd.dma_start(out=outB[:], in_=otB[0:RB, :])

    nc.compile()
    _prog_cache["p"] = nc
    return nc


def _solve_host(pA, pB, points):
    # pA: [NCORES, RA, 2, 5, 512], pB: [NCORES, RB, 2, 5, 512] bf16 products
    # rebuild full [5, 1014, XP] product maps (sobel-grid rows 0..1013)
    pA = pA.astype(np.float32).transpose(0, 3, 1, 2, 4)   # [c, 5, RA, 2, 512]
    pB = pB.astype(np.float32).transpose(0, 3, 1, 2, 4)
    pA = pA.reshape(NCORES, 5, RA, CW)[:, :, :, :XP]
    pB = pB.reshape(NCORES, 5, RB, CW)[:, :, :, :XP]
    nrows = (NCORES - 1) * BAND + BAND + PATCH - 1        # 1014
    full = np.empty((5, nrows, XP), np.float32)
    for c in range(NCORES):
        r0 = c * BAND
        take = BAND + PATCH - 1 if c == NCORES - 1 else BAND
        full[:, r0:r0 + min(RA, take)] = pA[c, :, :min(RA, take)]
        if take > RA:
            full[:, r0 + RA:r0 + take] = pB[c, :, :take - RA]
    # float64 integral image -> 15x15 box sums at the query points
    S = np.zeros((5, nrows + 1, XP + 1), np.float64)
    np.cumsum(full, axis=1, dtype=np.float64, out=S[:, 1:, 1:])
    np.cumsum(S[:, 1:, 1:], axis=2, out=S[:, 1:, 1:])
    xs = points[:, 0].astype(np.int64)
    ys = points[:, 1].astype(np.int64)
    box = (S[:, ys + PATCH, xs + PATCH] - S[:, ys, xs + PATCH]
           - S[:, ys + PATCH, xs] + S[:, ys, xs])        # [5, N]
    a, h01, d, b0, b1 = box
    det = a * d - h01 * h01
    dx = (d * b0 - h01 * b1) / det
    dy = (a * b1 - h01 * b0) / det
    return np.stack([dx, dy], axis=-1).astype(np.float32)


def _run(img1, img2, points, trace=False):
    in_maps = build_core_inputs(img1, img2)
    nc = build_program()
    res = run_bass_kernel_spmd(nc, in_maps, list(range(NCORES)), trace=trace)
    pA = np.stack([np.asarray(res.results[c]["outA"]).reshape(RA, 2, 5, 512)
                   for c in range(NCORES)])
    pB = np.stack([np.asarray(res.results[c]["outB"]).reshape(RB, 2, 5, 512)
                   for c in range(NCORES)])
    full = _solve_host(pA, pB, np.asarray(points))
    return full, res


def kernel(img1, img2, points1):
    full, _ = _run(np.asarray(img1), np.asarray(img2), np.asarray(points1))
    return full


# revision 25
# speedup vs baseline: 1.0001x; 1.0001x over previous
"""Lucas-Kanade delta_p kernel for 8 trn2 NeuronCores.

Strategy (dense per-pixel product maps, no on-device gather):
Every per-point output derives from 15x15 box-sums of five per-pixel
product maps (Ix^2, IxIy, Iy^2, Ix*E, Iy*E with E = img2-img1).  Points
lie in [0,1000)^2 so only the top-left ~1016x1016 corner matters.  Each
core owns a 125-row y-band (139 sobel rows incl. halo) and computes,
densely for all x:
 - full Sobel (vertical taps via banded lhsT, horizontal taps via
   shifted rhs views) as accumulating bf16 matmuls on the PE, split
   into a 116-row main tier and a 32-row bottom tier so no contraction
   exceeds 128 partitions
 - the five per-pixel product maps on ACT (squares) / DVE / GpSimd,
   written as bf16 directly into the output staging tile
The host finishes with a float64 2D integral image per map (the 15x15
box-sum) and the closed-form 2x2 solve at the 100k point locations.
No cross-core communication, no gather.
"""

import numpy as np
import ml_dtypes

import concourse.bass as bass
import concourse.bacc as bacc
import concourse.mybir as mybir
from concourse.tile import TileContext
from concourse.bass_utils import run_bass_kernel_spmd

F32 = mybir.dt.float32
BF16 = mybir.dt.bfloat16

NCORES = 8
BAND = 125          # output band rows per core
TA = 116            # main-tier image rows (sobel rows 0..113)
TB = 32             # bottom-tier image rows (img rows 114..145)
RA = 114            # valid sobel rows in tier A
RB = 25             # valid sobel rows in tier B (114..138)
IMG_ROWS = 146
CLD = 1040          # image columns loaded (shifted reads up to 1026)
CW = 1024           # working column width
XP = 1016           # product-map x columns that matter
PATCH = 15

AL = mybir.AluOpType
AF = mybir.ActivationFunctionType

# block offsets inside the packed weight tiles
_WA = {"smA": 0, "smAn": 128, "dfA": 256, "dfA2": 384}
_WB = {"smB": 0, "smBn": 32, "dfB": 64, "dfB2": 96}


def _packed_weights():
    sm = (2.0, 4.0, 2.0)
    df = (2.0, 0.0, -2.0)
    smA = np.zeros((128, 128), np.float32)   # sobel rows 0..113 from tier A
    dfA = np.zeros((128, 128), np.float32)
    for m in range(RA):
        for u in range(3):
            smA[m + u, m] = sm[u]
            dfA[m + u, m] = df[u]
    smB = np.zeros((32, 32), np.float32)     # sobel rows 114..138 from tier B
    dfB = np.zeros((32, 32), np.float32)
    for mB in range(RB):
        for u in range(3):
            smB[mB + u, mB] = sm[u]
            dfB[mB + u, mB] = df[u]
    wp = np.zeros((128, 640), np.float32)
    for nm, blk in (("smA", smA), ("smAn", -smA), ("dfA", dfA),
                    ("dfA2", 2.0 * dfA)):
        wp[:, _WA[nm]:_WA[nm] + 128] = blk
    for nm, blk in (("smB", smB), ("smBn", -smB), ("dfB", dfB),
                    ("dfB2", 2.0 * dfB)):
        wp[0:32, 512 + _WB[nm]:512 + _WB[nm] + 32] = blk
    return wp.astype(ml_dtypes.bfloat16)


def build_core_inputs(img1, img2):
    im1 = np.asarray(img1).reshape(img1.shape[-2], img1.shape[-1])
    im2 = np.asarray(img2).reshape(img2.shape[-2], img2.shape[-1])
    wp = _packed_weights()
    in_maps = []
    for c in range(NCORES):
        r0 = c * BAND
        in_maps.append(dict(
            img1b=np.ascontiguousarray(
                im1[r0:r0 + IMG_ROWS, :CLD].astype(ml_dtypes.bfloat16)),
            img2b=np.ascontiguousarray(
                im2[r0:r0 + IMG_ROWS, :CLD].astype(ml_dtypes.bfloat16)),
            wp=wp))
    return in_maps


_prog_cache = {}


def build_program():
    if "p" in _prog_cache:
        return _prog_cache["p"]
    nc = bacc.Bacc(None, target_bir_lowering=False, debug=True)
    img1b = nc.declare_dram_parameter("img1b", [IMG_ROWS, CLD], BF16, isOutput=False)
    img2b = nc.declare_dram_parameter("img2b", [IMG_ROWS, CLD], BF16, isOutput=False)
    wp_d = nc.declare_dram_parameter("wp", [128, 640], BF16, isOutput=False)
    # per-partition free layout: [chunk(2), map(5), 512]
    outA = nc.declare_dram_parameter("outA", [RA, 5120], BF16, isOutput=True)
    outB = nc.declare_dram_parameter("outB", [RB, 5120], BF16, isOutput=True)

    with TileContext(nc) as tc:
        with tc.tile_pool(name="cn", bufs=1) as cn, \
             tc.tile_pool(name="ps", bufs=8, space="PSUM") as ps:
            # ---- loads: everything bf16, sobel-critical first -----------
            i1A = cn.tile([TA, CLD], BF16, tag="i1A")
            i1B = cn.tile([TB, CLD], BF16, tag="i1B")
            i2A = cn.tile([TA, CLD], BF16, tag="i2A")
            i2B = cn.tile([TB, CLD], BF16, tag="i2B")
            wp = cn.tile([128, 640], BF16, tag="wp")
            # SP and ACT have hardware DGE queues; GpSimd's is software
            # (slow) so it never issues DMAs.  i1A/i1B/wp gate the matmuls.
            nc.sync.dma_start(out=i1A[:], in_=img1b[0:TA, :])
            nc.scalar.dma_start(out=wp[:], in_=wp_d[:])
            nc.sync.dma_start(out=i1B[:], in_=img1b[114:146, :])
            nc.scalar.dma_start(out=i2A[:], in_=img2b[0:TA, :])
            nc.sync.dma_start(out=i2B[:], in_=img2b[114:146, :])

            def WA(name):
                return wp[0:TA, _WA[name]:_WA[name] + TA]

            def WB(name):
                return wp[0:32, 512 + _WB[name]:512 + _WB[name] + 32]

            # ---- persistent SBUF tiles ---------------------------------
            IyAs = cn.tile([TA, CW], F32, tag="IyAs")
            IyBs = cn.tile([TB, CW], F32, tag="IyBs")
            EA = cn.tile([TA, CW], F32, tag="EA")
            EB = cn.tile([TB, CW], F32, tag="EB")
            otA = cn.tile([TA, 5120], BF16, tag="otA")
            otB = cn.tile([TB, 5120], BF16, tag="otB")

            for ic, c0 in enumerate((0, 512)):
                def shA(s):
                    return i1A[:, c0 + s:c0 + s + 512]

                def shB(s):
                    return i1B[:, c0 + s:c0 + s + 512]
                o = slice(c0, c0 + 512)
                # Sobel: Ix = vsm[c] - vsm[c+2]; Iy = vdf[c]+2vdf[c+1]+vdf[c+2]
                IxA = ps.tile([TA, 512], F32, tag="bank", name=f"IxA{ic}")
                nc.tensor.matmul(out=IxA[:], lhsT=WA("smA"), rhs=shA(0),
                                 start=True, stop=False)
                nc.tensor.matmul(out=IxA[:], lhsT=WA("smAn"), rhs=shA(2),
                                 start=False, stop=True)
                IxB = ps.tile([TB, 512], F32, tag="bank", name=f"IxB{ic}")
                nc.tensor.matmul(out=IxB[:], lhsT=WB("smB"), rhs=shB(0),
                                 start=True, stop=False)
                nc.tensor.matmul(out=IxB[:], lhsT=WB("smBn"), rhs=shB(2),
                                 start=False, stop=True)
                IyA = ps.tile([TA, 512], F32, tag="bank", name=f"IyA{ic}")
                nc.tensor.matmul(out=IyA[:], lhsT=WA("dfA"), rhs=shA(0),
                                 start=True, stop=False)
                nc.tensor.matmul(out=IyA[:], lhsT=WA("dfA2"), rhs=shA(1),
                                 start=False, stop=False)
                nc.tensor.matmul(out=IyA[:], lhsT=WA("dfA"), rhs=shA(2),
                                 start=False, stop=True)
                IyB = ps.tile([TB, 512], F32, tag="bank", name=f"IyB{ic}")
                nc.tensor.matmul(out=IyB[:], lhsT=WB("dfB"), rhs=shB(0),
                                 start=True, stop=False)
                nc.tensor.matmul(out=IyB[:], lhsT=WB("dfB2"), rhs=shB(1),
                                 start=False, stop=False)
                nc.tensor.matmul(out=IyB[:], lhsT=WB("dfB"), rhs=shB(2),
                                 start=False, stop=True)

                # E and Iy-to-SBUF
                nc.vector.tensor_tensor(out=EA[:, o], in0=i2A[:, o],
                                        in1=i1A[:, o], op=AL.subtract)
                nc.vector.tensor_tensor(out=EB[:, o], in0=i2B[:, o],
                                        in1=i1B[:, o], op=AL.subtract)
                nc.scalar.copy(out=IyAs[:, o], in_=IyA[:])
                nc.scalar.copy(out=IyBs[:, o], in_=IyB[:])

                # products straight into the bf16 staging tiles
                def dst(ott, ci):
                    base = ic * 2560 + ci * 512
                    return ott[:, base:base + 512]

                for tier, Ixp, Iys, Ep, ott in (
                        ("A", IxA, IyAs, EA, otA), ("B", IxB, IyBs, EB, otB)):
                    nc.scalar.activation(out=dst(ott, 0), in_=Ixp[:],
                                         func=AF.Square)
                    nc.scalar.activation(out=dst(ott, 2), in_=Iys[:, o],
                                         func=AF.Square)
                    nc.vector.tensor_tensor(out=dst(ott, 1), in0=Ixp[:],
                                            in1=Iys[:, o], op=AL.mult)
                    nc.vector.tensor_tensor(out=dst(ott, 3), in0=Ixp[:],
                                            in1=Ep[:, o], op=AL.mult)
                    eng4 = nc.gpsimd if tier == "A" else nc.vector
                    eng4.tensor_tensor(out=dst(ott, 4), in0=Iys[:, o],
                                       in1=Ep[:, o], op=AL.mult)

                oc = slice(ic * 2560, (ic + 1) * 2560)
                if ic == 0:
                    nc.sync.dma_start(out=outA[0:38, oc], in_=otA[0:38, oc])
                    nc.scalar.dma_start(out=outA[38:76, oc],
                                        in_=otA[38:76, oc])
                    nc.gpsimd.dma_start(out=outA[76:RA, oc],
                                        in_=otA[76:RA, oc])
                    nc.gpsimd.dma_start(out=outB[:, oc], in_=otB[0:RB, oc])
                else:
                    nc.sync.dma_start(out=outA[0:48, oc], in_=otA[0:48, oc])
                    nc.scalar.dma_start(out=outA[48:96, oc],
                                        in_=otA[48:96, oc])
                    nc.gpsimd.dma_start(out=outA[96:RA, oc],
                                        in_=otA[96:RA, oc])
                    nc.sync.dma_start(out=outB[:, oc], in_=otB[0:RB, oc])

    nc.compile()
    _prog_cache["p"] = nc
    return nc


def _solve_host(pA, pB, points):
    # pA: [NCORES, RA, 2, 5, 512], pB: [NCORES, RB, 2, 5, 512] bf16 products
    # rebuild full [5, 1014, XP] product maps (sobel-grid rows 0..1013)
    pA = pA.astype(np.float32).transpose(0, 3, 1, 2, 4)   # [c, 5, RA, 2, 512]
    pB = pB.astype(np.float32).transpose(0, 3, 1, 2, 4)
    pA = pA.reshape(NCORES, 5, RA, CW)[:, :, :, :XP]
    pB = pB.reshape(NCORES, 5, RB, CW)[:, :, :, :XP]
    nrows = (NCORES - 1) * BAND + BAND + PATCH - 1        # 1014
    full = np.empty((5, nrows, XP), np.float32)
    for c in range(NCORES):
        r0 = c * BAND
        take = BAND + PATCH - 1 if c == NCORES - 1 else BAND
        full[:, r0:r0 + min(RA, take)] = pA[c, :, :min(RA, take)]
        if take > RA:
            full[:, r0 + RA:r0 + take] = pB[c, :, :take - RA]
    # float64 integral image -> 15x15 box sums at the query points
    S = np.zeros((5, nrows + 1, XP + 1), np.float64)
    np.cumsum(full, axis=1, dtype=np.float64, out=S[:, 1:, 1:])
    np.cumsum(S[:, 1:, 1:], axis=2, out=S[:, 1:, 1:])
    xs = points[:, 0].astype(np.int64)
    ys = points[:, 1].astype(np.int64)
    box = (S[:, ys + PATCH, xs + PATCH] - S[:, ys, xs + PATCH]
           - S[:, ys + PATCH, xs] + S[:, ys, xs])        # [5, N]
    a, h01, d, b0, b1 = box
    det = a * d - h01 * h01
    dx = (d * b0 - h01 * b1) / det
    dy = (a * b1 - h01 * b0) / det
    return np.stack([dx, dy], axis=-1).astype(np.float32)


def _run(img1, img2, points, trace=False):
    in_maps = build_core_inputs(img1, img2)
    nc = build_program()
    res = run_bass_kernel_spmd(nc, in_maps, list(range(NCORES)), trace=trace)
    pA = np.stack([np.asarray(res.results[c]["outA"]).reshape(RA, 2, 5, 512)
                   for c in range(NCORES)])
    pB = np.stack([np.asarray(res.results[c]["outB"]).reshape(RB, 2, 5, 512)
                   for c in range(NCORES)])
    full = _solve_host(pA, pB, np.asarray(points))
    return full, res


def kernel(img1, img2, points1):
    full, _ = _run(np.asarray(img1), np.asarray(img2), np.asarray(points1))
    return full


# revision 26
# speedup vs baseline: 1.1898x; 1.1897x over previous
"""Lucas-Kanade delta_p kernel for 8 trn2 NeuronCores.

Strategy (dense per-pixel product maps, no on-device gather):
Every per-point output derives from 15x15 box-sums of five per-pixel
product maps (Ix^2, IxIy, Iy^2, Ix*E, Iy*E with E = img2-img1).  Points
lie in [0,1000)^2 so only the top-left ~1016x1016 corner matters.  Each
core owns a 125-row y-band (139 sobel rows incl. halo) and computes,
densely for all x:
 - full Sobel (vertical taps via banded lhsT, horizontal taps via
   shifted rhs views) as accumulating bf16 matmuls on the PE, split
   into a 116-row main tier and a 32-row bottom tier so no contraction
   exceeds 128 partitions
 - the five per-pixel product maps on ACT (squares) / DVE / GpSimd,
   written as bf16 directly into the output staging tile
The host finishes with a float64 2D integral image per map (the 15x15
box-sum) and the closed-form 2x2 solve at the 100k point locations.
No cross-core communication, no gather.
"""

import numpy as np
import ml_dtypes

import concourse.bass as bass
import concourse.bacc as bacc
import concourse.mybir as mybir
from concourse.tile import TileContext
from concourse.bass_utils import run_bass_kernel_spmd

F32 = mybir.dt.float32
BF16 = mybir.dt.bfloat16

NCORES = 8
BAND = 125          # output band rows per core
TA = 116            # main-tier image rows (sobel rows 0..113)
TB = 32             # bottom-tier image rows (img rows 114..145)
RA = 114            # valid sobel rows in tier A
RB = 25             # valid sobel rows in tier B (114..138)
IMG_ROWS = 146
CLD = 1040          # image columns loaded (shifted reads up to 1026)
CW = 1024           # working column width
XP = 1016           # product-map x columns that matter
PATCH = 15

AL = mybir.AluOpType
AF = mybir.ActivationFunctionType

# block offsets inside the packed weight tiles
_WA = {"smA": 0, "smAn": 128, "dfA": 256, "dfA2": 384}
_WB = {"smB": 0, "smBn": 32, "dfB": 64, "dfB2": 96}


def _packed_weights():
    sm = (2.0, 4.0, 2.0)
    df = (2.0, 0.0, -2.0)
    smA = np.zeros((128, 128), np.float32)   # sobel rows 0..113 from tier A
    dfA = np.zeros((128, 128), np.float32)
    for m in range(RA):
        for u in range(3):
            smA[m + u, m] = sm[u]
            dfA[m + u, m] = df[u]
    smB = np.zeros((32, 32), np.float32)     # sobel rows 114..138 from tier B
    dfB = np.zeros((32, 32), np.float32)
    for mB in range(RB):
        for u in range(3):
            smB[mB + u, mB] = sm[u]
            dfB[mB + u, mB] = df[u]
    wp = np.zeros((128, 640), np.float32)
    for nm, blk in (("smA", smA), ("smAn", -smA), ("dfA", dfA),
                    ("dfA2", 2.0 * dfA)):
        wp[:, _WA[nm]:_WA[nm] + 128] = blk
    for nm, blk in (("smB", smB), ("smBn", -smB), ("dfB", dfB),
                    ("dfB2", 2.0 * dfB)):
        wp[0:32, 512 + _WB[nm]:512 + _WB[nm] + 32] = blk
    return wp.astype(ml_dtypes.bfloat16)


def build_core_inputs(img1, img2):
    im1 = np.asarray(img1).reshape(img1.shape[-2], img1.shape[-1])
    im2 = np.asarray(img2).reshape(img2.shape[-2], img2.shape[-1])
    wp = _packed_weights()
    in_maps = []
    for c in range(NCORES):
        r0 = c * BAND
        in_maps.append(dict(
            img1b=np.ascontiguousarray(
                im1[r0:r0 + IMG_ROWS, :CLD].astype(ml_dtypes.bfloat16)),
            img2b=np.ascontiguousarray(
                im2[r0:r0 + IMG_ROWS, :CLD].astype(ml_dtypes.bfloat16)),
            wp=wp))
    return in_maps


_prog_cache = {}


def build_program():
    if "p" in _prog_cache:
        return _prog_cache["p"]
    nc = bacc.Bacc(None, target_bir_lowering=False, debug=True)
    img1b = nc.declare_dram_parameter("img1b", [IMG_ROWS, CLD], BF16, isOutput=False)
    img2b = nc.declare_dram_parameter("img2b", [IMG_ROWS, CLD], BF16, isOutput=False)
    wp_d = nc.declare_dram_parameter("wp", [128, 640], BF16, isOutput=False)
    # per-partition free layout: [chunk(2), map(5), 512]
    outA = nc.declare_dram_parameter("outA", [RA, 5120], BF16, isOutput=True)
    outB = nc.declare_dram_parameter("outB", [RB, 5120], BF16, isOutput=True)

    with TileContext(nc) as tc:
        with tc.tile_pool(name="cn", bufs=1) as cn, \
             tc.tile_pool(name="ps", bufs=8, space="PSUM") as ps:
            # ---- loads: everything bf16, sobel-critical first -----------
            i1A = cn.tile([TA, CLD], BF16, tag="i1A")
            i1B = cn.tile([TB, CLD], BF16, tag="i1B")
            i2A = cn.tile([TA, CLD], BF16, tag="i2A")
            i2B = cn.tile([TB, CLD], BF16, tag="i2B")
            wp = cn.tile([128, 640], BF16, tag="wp")
            # SP and ACT have hardware DGE queues; GpSimd's is software
            # (slow) so it never issues DMAs.  i1A/i1B/wp gate the matmuls.
            nc.sync.dma_start(out=i1A[:], in_=img1b[0:TA, :])
            nc.scalar.dma_start(out=wp[:], in_=wp_d[:])
            nc.sync.dma_start(out=i1B[:], in_=img1b[114:146, :])
            nc.scalar.dma_start(out=i2A[:], in_=img2b[0:TA, :])
            nc.sync.dma_start(out=i2B[:], in_=img2b[114:146, :])

            def WA(name):
                return wp[0:TA, _WA[name]:_WA[name] + TA]

            def WB(name):
                return wp[0:32, 512 + _WB[name]:512 + _WB[name] + 32]

            # ---- persistent SBUF tiles ---------------------------------
            IyAs = cn.tile([TA, CW], F32, tag="IyAs")
            IyBs = cn.tile([TB, CW], F32, tag="IyBs")
            EA = cn.tile([TA, CW], F32, tag="EA")
            EB = cn.tile([TB, CW], F32, tag="EB")
            otA = cn.tile([TA, 5120], BF16, tag="otA")
            otB = cn.tile([TB, 5120], BF16, tag="otB")

            for ic, c0 in enumerate((0, 512)):
                def shA(s):
                    return i1A[:, c0 + s:c0 + s + 512]

                def shB(s):
                    return i1B[:, c0 + s:c0 + s + 512]
                o = slice(c0, c0 + 512)
                # Sobel: Ix = vsm[c] - vsm[c+2]; Iy = vdf[c]+2vdf[c+1]+vdf[c+2]
                IxA = ps.tile([TA, 512], F32, tag="bank", name=f"IxA{ic}")
                nc.tensor.matmul(out=IxA[:], lhsT=WA("smA"), rhs=shA(0),
                                 start=True, stop=False)
                nc.tensor.matmul(out=IxA[:], lhsT=WA("smAn"), rhs=shA(2),
                                 start=False, stop=True)
                IxB = ps.tile([TB, 512], F32, tag="bank", name=f"IxB{ic}")
                nc.tensor.matmul(out=IxB[:], lhsT=WB("smB"), rhs=shB(0),
                                 start=True, stop=False)
                nc.tensor.matmul(out=IxB[:], lhsT=WB("smBn"), rhs=shB(2),
                                 start=False, stop=True)
                IyA = ps.tile([TA, 512], F32, tag="bank", name=f"IyA{ic}")
                nc.tensor.matmul(out=IyA[:], lhsT=WA("dfA"), rhs=shA(0),
                                 start=True, stop=False)
                nc.tensor.matmul(out=IyA[:], lhsT=WA("dfA2"), rhs=shA(1),
                                 start=False, stop=False)
                nc.tensor.matmul(out=IyA[:], lhsT=WA("dfA"), rhs=shA(2),
                                 start=False, stop=True)
                IyB = ps.tile([TB, 512], F32, tag="bank", name=f"IyB{ic}")
                nc.tensor.matmul(out=IyB[:], lhsT=WB("dfB"), rhs=shB(0),
                                 start=True, stop=False)
                nc.tensor.matmul(out=IyB[:], lhsT=WB("dfB2"), rhs=shB(1),
                                 start=False, stop=False)
                nc.tensor.matmul(out=IyB[:], lhsT=WB("dfB"), rhs=shB(2),
                                 start=False, stop=True)

                # E and Iy-to-SBUF
                nc.vector.tensor_tensor(out=EA[:, o], in0=i2A[:, o],
                                        in1=i1A[:, o], op=AL.subtract)
                nc.vector.tensor_tensor(out=EB[:, o], in0=i2B[:, o],
                                        in1=i1B[:, o], op=AL.subtract)
                nc.scalar.copy(out=IyAs[:, o], in_=IyA[:])
                nc.scalar.copy(out=IyBs[:, o], in_=IyB[:])

                # products straight into the bf16 staging tiles
                def dst(ott, ci):
                    base = ic * 2560 + ci * 512
                    return ott[:, base:base + 512]

                for tier, Ixp, Iys, Ep, ott in (
                        ("A", IxA, IyAs, EA, otA), ("B", IxB, IyBs, EB, otB)):
                    nc.scalar.activation(out=dst(ott, 0), in_=Ixp[:],
                                         func=AF.Square)
                    nc.scalar.activation(out=dst(ott, 2), in_=Iys[:, o],
                                         func=AF.Square)
                    nc.vector.tensor_tensor(out=dst(ott, 1), in0=Ixp[:],
                                            in1=Iys[:, o], op=AL.mult)
                    nc.vector.tensor_tensor(out=dst(ott, 3), in0=Ixp[:],
                                            in1=Ep[:, o], op=AL.mult)
                    eng4 = nc.gpsimd if tier == "A" else nc.vector
                    eng4.tensor_tensor(out=dst(ott, 4), in0=Iys[:, o],
                                       in1=Ep[:, o], op=AL.mult)

                oc = slice(ic * 2560, (ic + 1) * 2560)
                if ic == 0:
                    nc.sync.dma_start(out=outA[:, oc], in_=otA[0:RA, oc])
                else:
                    nc.scalar.dma_start(out=outA[0:70, oc], in_=otA[0:70, oc])
                    nc.sync.dma_start(out=outA[70:RA, oc],
                                      in_=otA[70:RA, oc])
                    nc.gpsimd.dma_start(out=outB[:], in_=otB[0:RB, :])

    nc.compile()
    _prog_cache["p"] = nc
    return nc


def _solve_host(pA, pB, points):
    # pA: [NCORES, RA, 2, 5, 512], pB: [NCORES, RB, 2, 5, 512] bf16 products
    # rebuild full [5, 1014, XP] product maps (sobel-grid rows 0..1013)
    pA = pA.astype(np.float32).transpose(0, 3, 1, 2, 4)   # [c, 5, RA, 2, 512]
    pB = pB.astype(np.float32).transpose(0, 3, 1, 2, 4)
    pA = pA.reshape(NCORES, 5, RA, CW)[:, :, :, :XP]
    pB = pB.reshape(NCORES, 5, RB, CW)[:, :, :, :XP]
    nrows = (NCORES - 1) * BAND + BAND + PATCH - 1        # 1014
    full = np.empty((5, nrows, XP), np.float32)
    for c in range(NCORES):
        r0 = c * BAND
        take = BAND + PATCH - 1 if c == NCORES - 1 else BAND
        full[:, r0:r0 + min(RA, take)] = pA[c, :, :min(RA, take)]
        if take > RA:
            full[:, r0 + RA:r0 + take] = pB[c, :, :take - RA]
    # float64 integral image -> 15x15 box sums at the query points
    S = np.zeros((5, nrows + 1, XP + 1), np.float64)
    np.cumsum(full, axis=1, dtype=np.float64, out=S[:, 1:, 1:])
    np.cumsum(S[:, 1:, 1:], axis=2, out=S[:, 1:, 1:])
    xs = points[:, 0].astype(np.int64)
    ys = points[:, 1].astype(np.int64)
    box = (S[:, ys + PATCH, xs + PATCH] - S[:, ys, xs + PATCH]
           - S[:, ys + PATCH, xs] + S[:, ys, xs])        # [5, N]
    a, h01, d, b0, b1 = box
    det = a * d - h01 * h01
    dx = (d * b0 - h01 * b1) / det
    dy = (a * b1 - h01 * b0) / det
    return np.stack([dx, dy], axis=-1).astype(np.float32)


def _run(img1, img2, points, trace=False):
    in_maps = build_core_inputs(img1, img2)
    nc = build_program()
    res = run_bass_kernel_spmd(nc, in_maps, list(range(NCORES)), trace=trace)
    pA = np.stack([np.asarray(res.results[c]["outA"]).reshape(RA, 2, 5, 512)
                   for c in range(NCORES)])
    pB = np.stack([np.asarray(res.results[c]["outB"]).reshape(RB, 2, 5, 512)
                   for c in range(NCORES)])
    full = _solve_host(pA, pB, np.asarray(points))
    return full, res


def kernel(img1, img2, points1):
    full, _ = _run(np.asarray(img1), np.asarray(img2), np.asarray(points1))
    return full


# revision 27
# speedup vs baseline: 1.5763x; 1.3248x over previous
"""Lucas-Kanade delta_p kernel for 8 trn2 NeuronCores.

Strategy (dense per-pixel product maps, no on-device gather):
Every per-point output derives from 15x15 box-sums of five per-pixel
product maps (Ix^2, IxIy, Iy^2, Ix*E, Iy*E with E = img2-img1).  Points
lie in [0,1000)^2 so only the top-left ~1016x1016 corner matters.  The
box-sums are evaluated on the host from an integral image, so the cores
produce DISJOINT row bands of the product maps (no halo): each of the 8
cores computes 126 sobel rows from a 128-row image slice:
 - full Sobel (vertical taps via banded lhsT, horizontal taps via
   shifted rhs views) as accumulating bf16 matmuls on the PE
 - the five per-pixel product maps on ACT (squares) / DVE / GpSimd,
   written as bf16 directly into the output staging tile
The host computes the last 6 product rows (1008..1013) directly in
numpy, builds a float64 integral image per map, and finishes with the
closed-form 2x2 solve at the 100k point locations.  No cross-core
communication, no gather.
"""

import numpy as np
import ml_dtypes

import concourse.bass as bass
import concourse.bacc as bacc
import concourse.mybir as mybir
from concourse.tile import TileContext
from concourse.bass_utils import run_bass_kernel_spmd

F32 = mybir.dt.float32
BF16 = mybir.dt.bfloat16

NCORES = 8
BAND = 126          # sobel/product rows per core (disjoint)
TA = 128            # image rows loaded per core
NROWS = 1014        # product rows needed globally (y+u <= 999+14)
CLD = 1040          # image columns loaded (shifted reads up to 1026)
CW = 1024           # working column width
XP = 1016           # product-map x columns that matter
PATCH = 15

AL = mybir.AluOpType
AF = mybir.ActivationFunctionType

_WO = {"sm": 0, "smn": 128, "df": 256, "df2": 384}


def _packed_weights():
    sm = (2.0, 4.0, 2.0)
    df = (2.0, 0.0, -2.0)
    smA = np.zeros((128, BAND), np.float32)
    dfA = np.zeros((128, BAND), np.float32)
    for m in range(BAND):
        for u in range(3):
            smA[m + u, m] = sm[u]
            dfA[m + u, m] = df[u]
    wp = np.zeros((128, 512), np.float32)
    for nm, blk in (("sm", smA), ("smn", -smA), ("df", dfA),
                    ("df2", 2.0 * dfA)):
        wp[:, _WO[nm]:_WO[nm] + BAND] = blk
    return wp.astype(ml_dtypes.bfloat16)


def build_core_inputs(img1, img2):
    im1 = np.asarray(img1).reshape(img1.shape[-2], img1.shape[-1])
    im2 = np.asarray(img2).reshape(img2.shape[-2], img2.shape[-1])
    wp = _packed_weights()
    in_maps = []
    for c in range(NCORES):
        r0 = c * BAND
        in_maps.append(dict(
            img1b=np.ascontiguousarray(
                im1[r0:r0 + TA, :CLD].astype(ml_dtypes.bfloat16)),
            img2b=np.ascontiguousarray(
                im2[r0:r0 + TA, :CLD].astype(ml_dtypes.bfloat16)),
            wp=wp))
    return in_maps


_prog_cache = {}


def build_program():
    if "p" in _prog_cache:
        return _prog_cache["p"]
    nc = bacc.Bacc(None, target_bir_lowering=False, debug=True)
    img1b = nc.declare_dram_parameter("img1b", [TA, CLD], BF16, isOutput=False)
    img2b = nc.declare_dram_parameter("img2b", [TA, CLD], BF16, isOutput=False)
    wp_d = nc.declare_dram_parameter("wp", [128, 512], BF16, isOutput=False)
    # per-partition free layout: [chunk(2), map(5), 512]
    outA = nc.declare_dram_parameter("outA", [BAND, 5120], BF16, isOutput=True)

    with TileContext(nc) as tc:
        with tc.tile_pool(name="cn", bufs=1) as cn, \
             tc.tile_pool(name="ps", bufs=8, space="PSUM") as ps:
            # ---- loads ---------------------------------------------------
            i1A = cn.tile([TA, CLD], BF16, tag="i1A")
            i2A = cn.tile([TA, CLD], BF16, tag="i2A")
            wp = cn.tile([128, 512], BF16, tag="wp")
            nc.sync.dma_start(out=i1A[:], in_=img1b[:])
            nc.scalar.dma_start(out=wp[:], in_=wp_d[:])
            nc.scalar.dma_start(out=i2A[:], in_=img2b[:])

            def W(name):
                return wp[:, _WO[name]:_WO[name] + BAND]

            # ---- persistent SBUF tiles ----------------------------------
            IyAs = cn.tile([BAND, CW], F32, tag="IyAs")
            EA = cn.tile([BAND, CW], F32, tag="EA")
            ot = cn.tile([BAND, 5120], BF16, tag="ot")

            for ic, c0 in enumerate((0, 512)):
                def sh(s):
                    return i1A[:, c0 + s:c0 + s + 512]
                o = slice(c0, c0 + 512)
                # Sobel: Ix = vsm[c] - vsm[c+2]; Iy = vdf[c]+2vdf[c+1]+vdf[c+2]
                Ix = ps.tile([BAND, 512], F32, tag="bank", name=f"Ix{ic}")
                nc.tensor.matmul(out=Ix[:], lhsT=W("sm"), rhs=sh(0),
                                 start=True, stop=False)
                nc.tensor.matmul(out=Ix[:], lhsT=W("smn"), rhs=sh(2),
                                 start=False, stop=True)
                Iy = ps.tile([BAND, 512], F32, tag="bank", name=f"Iy{ic}")
                nc.tensor.matmul(out=Iy[:], lhsT=W("df"), rhs=sh(0),
                                 start=True, stop=False)
                nc.tensor.matmul(out=Iy[:], lhsT=W("df2"), rhs=sh(1),
                                 start=False, stop=False)
                nc.tensor.matmul(out=Iy[:], lhsT=W("df"), rhs=sh(2),
                                 start=False, stop=True)

                nc.vector.tensor_tensor(out=EA[:, o], in0=i2A[0:BAND, o],
                                        in1=i1A[0:BAND, o], op=AL.subtract)
                nc.scalar.copy(out=IyAs[:, o], in_=Iy[:])

                # products straight into the bf16 staging tile
                def dst(ci):
                    base = ic * 2560 + ci * 512
                    return ot[:, base:base + 512]

                nc.scalar.activation(out=dst(0), in_=Ix[:], func=AF.Square)
                nc.scalar.activation(out=dst(2), in_=IyAs[:, o],
                                     func=AF.Square)
                nc.vector.tensor_tensor(out=dst(1), in0=Ix[:],
                                        in1=IyAs[:, o], op=AL.mult)
                nc.vector.tensor_tensor(out=dst(3), in0=Ix[:],
                                        in1=EA[:, o], op=AL.mult)
                nc.gpsimd.tensor_tensor(out=dst(4), in0=IyAs[:, o],
                                        in1=EA[:, o], op=AL.mult)

                oc = slice(ic * 2560, (ic + 1) * 2560)
                if ic == 0:
                    nc.sync.dma_start(out=outA[:, oc], in_=ot[:, oc])
                else:
                    nc.scalar.dma_start(out=outA[0:63, oc], in_=ot[0:63, oc])
                    nc.sync.dma_start(out=outA[63:BAND, oc],
                                      in_=ot[63:BAND, oc])

    nc.compile()
    _prog_cache["p"] = nc
    return nc


def _host_tail_products(im1, im2):
    """Product-map rows 1008..1013 (not covered by the 8 cores), float64."""
    r0, r1 = NCORES * BAND, NROWS
    need = r1 - r0                         # 6 rows
    a = im1[r0:r1 + 2, :XP + 2].astype(np.float64)
    b = im2[r0:r1, :XP].astype(np.float64)
    sm = np.array([2.0, 4.0, 2.0])
    df = np.array([2.0, 0.0, -2.0])
    vs = sum(sm[u] * a[u:u + need] for u in range(3))
    vd = sum(df[u] * a[u:u + need] for u in range(3))
    ix = vs[:, 0:XP] - vs[:, 2:XP + 2]
    t = vd[:, 0:XP + 1] + vd[:, 1:XP + 2]
    iy = t[:, 0:XP] + t[:, 1:XP + 1]
    e = b - im1[r0:r1, :XP].astype(np.float64)
    return np.stack([ix * ix, ix * iy, iy * iy, ix * e, iy * e])


def _solve_host(pA, img1, img2, points):
    # pA: [NCORES, BAND, 2, 5, 512] bf16 products -> full [5, NROWS, XP]
    pA = pA.astype(np.float32).transpose(0, 3, 1, 2, 4)   # [c, 5, BAND, 2, 512]
    pA = pA.reshape(NCORES, 5, BAND, CW)[:, :, :, :XP]
    full = np.empty((5, NROWS, XP), np.float32)
    full[:, :NCORES * BAND] = pA.transpose(1, 0, 2, 3).reshape(
        5, NCORES * BAND, XP)
    im1 = np.asarray(img1).reshape(img1.shape[-2], img1.shape[-1])
    im2 = np.asarray(img2).reshape(img2.shape[-2], img2.shape[-1])
    full[:, NCORES * BAND:] = _host_tail_products(im1, im2)
    # float64 integral image -> 15x15 box sums at the query points
    S = np.zeros((5, NROWS + 1, XP + 1), np.float64)
    np.cumsum(full, axis=1, dtype=np.float64, out=S[:, 1:, 1:])
    np.cumsum(S[:, 1:, 1:], axis=2, out=S[:, 1:, 1:])
    xs = points[:, 0].astype(np.int64)
    ys = points[:, 1].astype(np.int64)
    box = (S[:, ys + PATCH, xs + PATCH] - S[:, ys, xs + PATCH]
           - S[:, ys + PATCH, xs] + S[:, ys, xs])        # [5, N]
    a, h01, d, b0, b1 = box
    det = a * d - h01 * h01
    dx = (d * b0 - h01 * b1) / det
    dy = (a * b1 - h01 * b0) / det
    return np.stack([dx, dy], axis=-1).astype(np.float32)


def _run(img1, img2, points, trace=False):
    in_maps = build_core_inputs(img1, img2)
    nc = build_program()
    res = run_bass_kernel_spmd(nc, in_maps, list(range(NCORES)), trace=trace)
    pA = np.stack([np.asarray(res.results[c]["outA"]).reshape(BAND, 2, 5, 512)
                   for c in range(NCORES)])
    full = _solve_host(pA, img1, img2, np.asarray(points))
    return full, res


def kernel(img1, img2, points1):
    full, _ = _run(np.asarray(img1), np.asarray(img2), np.asarray(points1))
    return full


# revision 29
# speedup vs baseline: 1.6109x; 1.0219x over previous
"""Lucas-Kanade delta_p kernel for 8 trn2 NeuronCores.

Strategy (dense per-pixel product maps, no on-device gather):
Every per-point output derives from 15x15 box-sums of five per-pixel
product maps (Ix^2, IxIy, Iy^2, Ix*E, Iy*E with E = img2-img1).  Points
lie in [0,1000)^2 so only the top-left ~1016x1016 corner matters.  The
box-sums are evaluated on the host from an integral image, so the cores
produce DISJOINT row bands of the product maps (no halo): each of the 8
cores computes 126 sobel rows from a 128-row image slice:
 - full Sobel (vertical taps via banded lhsT, horizontal taps via
   shifted rhs views) as accumulating bf16 matmuls on the PE
 - the five per-pixel product maps on ACT (squares) / DVE / GpSimd,
   written as bf16 directly into the output staging tile
The host computes the last 6 product rows (1008..1013) directly in
numpy, builds a float64 integral image per map, and finishes with the
closed-form 2x2 solve at the 100k point locations.  No cross-core
communication, no gather.
"""

import numpy as np
import ml_dtypes

import concourse.bass as bass
import concourse.bacc as bacc
import concourse.mybir as mybir
from concourse.tile import TileContext
from concourse.bass_utils import run_bass_kernel_spmd

F32 = mybir.dt.float32
BF16 = mybir.dt.bfloat16

NCORES = 8
BAND = 126          # sobel/product rows per core (disjoint)
TA = 128            # image rows loaded per core
NROWS = 1014        # product rows needed globally (y+u <= 999+14)
CLD = 1040          # image columns loaded (shifted reads up to 1026)
CW = 1024           # working column width
XP = 1016           # product-map x columns that matter
PATCH = 15

AL = mybir.AluOpType
AF = mybir.ActivationFunctionType

_WO = {"sm": 0, "smn": 128, "df": 256, "df2": 384}


def _packed_weights():
    sm = (2.0, 4.0, 2.0)
    df = (2.0, 0.0, -2.0)
    smA = np.zeros((128, BAND), np.float32)
    dfA = np.zeros((128, BAND), np.float32)
    for m in range(BAND):
        for u in range(3):
            smA[m + u, m] = sm[u]
            dfA[m + u, m] = df[u]
    wp = np.zeros((128, 512), np.float32)
    for nm, blk in (("sm", smA), ("smn", -smA), ("df", dfA),
                    ("df2", 2.0 * dfA)):
        wp[:, _WO[nm]:_WO[nm] + BAND] = blk
    return wp.astype(ml_dtypes.bfloat16)


def build_core_inputs(img1, img2):
    im1 = np.asarray(img1).reshape(img1.shape[-2], img1.shape[-1])
    im2 = np.asarray(img2).reshape(img2.shape[-2], img2.shape[-1])
    wp = _packed_weights()
    in_maps = []
    for c in range(NCORES):
        r0 = c * BAND
        in_maps.append(dict(
            img1b=np.ascontiguousarray(
                im1[r0:r0 + TA, :CLD].astype(ml_dtypes.bfloat16)),
            img2b=np.ascontiguousarray(
                im2[r0:r0 + TA, :CLD].astype(ml_dtypes.bfloat16)),
            wp=wp))
    return in_maps


_prog_cache = {}


def build_program():
    if "p" in _prog_cache:
        return _prog_cache["p"]
    nc = bacc.Bacc(None, target_bir_lowering=False, debug=True)
    img1b = nc.declare_dram_parameter("img1b", [TA, CLD], BF16, isOutput=False)
    img2b = nc.declare_dram_parameter("img2b", [TA, CLD], BF16, isOutput=False)
    wp_d = nc.declare_dram_parameter("wp", [128, 512], BF16, isOutput=False)
    # per-partition free layout: [chunk(2), map(5), 512]
    outA = nc.declare_dram_parameter("outA", [BAND, 5120], BF16, isOutput=True)

    with TileContext(nc) as tc:
        with tc.tile_pool(name="cn", bufs=1) as cn, \
             tc.tile_pool(name="ps", bufs=8, space="PSUM") as ps:
            # ---- loads ---------------------------------------------------
            i1A = cn.tile([TA, CLD], BF16, tag="i1A")
            i2A = cn.tile([TA, CLD], BF16, tag="i2A")
            wp = cn.tile([128, 512], BF16, tag="wp")
            nc.sync.dma_start(out=i1A[:], in_=img1b[:])
            nc.scalar.dma_start(out=wp[:], in_=wp_d[:])
            nc.scalar.dma_start(out=i2A[:], in_=img2b[:])

            def W(name):
                return wp[:, _WO[name]:_WO[name] + BAND]

            # ---- persistent SBUF tiles ----------------------------------
            IyAs = cn.tile([BAND, CW], F32, tag="IyAs")
            EA = cn.tile([BAND, CW], F32, tag="EA")
            ot = cn.tile([BAND, 5120], BF16, tag="ot")

            CHUNKS = ((0, 384), (384, 384), (768, 256))
            OTOFF = (0, 1920, 3840)
            for ic, (c0, cw) in enumerate(CHUNKS):
                def sh(s):
                    return i1A[:, c0 + s:c0 + s + cw]
                o = slice(c0, c0 + cw)
                # Sobel: Ix = vsm[c] - vsm[c+2]; Iy = vdf[c]+2vdf[c+1]+vdf[c+2]
                Ix = ps.tile([BAND, cw], F32, tag="bank", name=f"Ix{ic}")
                nc.tensor.matmul(out=Ix[:], lhsT=W("sm"), rhs=sh(0),
                                 start=True, stop=False)
                nc.tensor.matmul(out=Ix[:], lhsT=W("smn"), rhs=sh(2),
                                 start=False, stop=True)
                Iy = ps.tile([BAND, cw], F32, tag="bank", name=f"Iy{ic}")
                nc.tensor.matmul(out=Iy[:], lhsT=W("df"), rhs=sh(0),
                                 start=True, stop=False)
                nc.tensor.matmul(out=Iy[:], lhsT=W("df2"), rhs=sh(1),
                                 start=False, stop=False)
                nc.tensor.matmul(out=Iy[:], lhsT=W("df"), rhs=sh(2),
                                 start=False, stop=True)

                nc.vector.tensor_tensor(out=EA[:, o], in0=i2A[0:BAND, o],
                                        in1=i1A[0:BAND, o], op=AL.subtract)
                nc.scalar.copy(out=IyAs[:, o], in_=Iy[:])

                # products straight into the bf16 staging tile
                def dst(ci):
                    base = OTOFF[ic] + ci * cw
                    return ot[:, base:base + cw]

                nc.scalar.activation(out=dst(0), in_=Ix[:], func=AF.Square)
                nc.scalar.activation(out=dst(2), in_=IyAs[:, o],
                                     func=AF.Square)
                nc.vector.tensor_tensor(out=dst(1), in0=Ix[:],
                                        in1=IyAs[:, o], op=AL.mult)
                nc.vector.tensor_tensor(out=dst(3), in0=Ix[:],
                                        in1=EA[:, o], op=AL.mult)
                nc.gpsimd.tensor_tensor(out=dst(4), in0=IyAs[:, o],
                                        in1=EA[:, o], op=AL.mult)

                oc = slice(OTOFF[ic], OTOFF[ic] + 5 * cw)
                if ic == 0:
                    nc.sync.dma_start(out=outA[:, oc], in_=ot[:, oc])
                elif ic == 1:
                    nc.scalar.dma_start(out=outA[:, oc], in_=ot[:, oc])
                else:
                    nc.sync.dma_start(out=outA[0:63, oc], in_=ot[0:63, oc])
                    nc.scalar.dma_start(out=outA[63:BAND, oc],
                                        in_=ot[63:BAND, oc])

    nc.compile()
    _prog_cache["p"] = nc
    return nc


def _host_tail_products(im1, im2):
    """Product-map rows 1008..1013 (not covered by the 8 cores), float64."""
    r0, r1 = NCORES * BAND, NROWS
    need = r1 - r0                         # 6 rows
    a = im1[r0:r1 + 2, :XP + 2].astype(np.float64)
    b = im2[r0:r1, :XP].astype(np.float64)
    sm = np.array([2.0, 4.0, 2.0])
    df = np.array([2.0, 0.0, -2.0])
    vs = sum(sm[u] * a[u:u + need] for u in range(3))
    vd = sum(df[u] * a[u:u + need] for u in range(3))
    ix = vs[:, 0:XP] - vs[:, 2:XP + 2]
    t = vd[:, 0:XP + 1] + vd[:, 1:XP + 2]
    iy = t[:, 0:XP] + t[:, 1:XP + 1]
    e = b - im1[r0:r1, :XP].astype(np.float64)
    return np.stack([ix * ix, ix * iy, iy * iy, ix * e, iy * e])


_CHUNKS = ((0, 384), (384, 384), (768, 256))
_OTOFF = (0, 1920, 3840)


def _solve_host(pA, img1, img2, points):
    # pA: [NCORES, BAND, 5120] bf16; per row: [5, cw] per chunk, concat
    pA = pA.astype(np.float32)
    maps = np.empty((NCORES, BAND, 5, CW), np.float32)
    for (c0, cw), off in zip(_CHUNKS, _OTOFF):
        blk = pA[:, :, off:off + 5 * cw].reshape(NCORES, BAND, 5, cw)
        maps[:, :, :, c0:c0 + cw] = blk
    full = np.empty((5, NROWS, XP), np.float32)
    full[:, :NCORES * BAND] = maps[:, :, :, :XP].transpose(2, 0, 1, 3).reshape(
        5, NCORES * BAND, XP)
    im1 = np.asarray(img1).reshape(img1.shape[-2], img1.shape[-1])
    im2 = np.asarray(img2).reshape(img2.shape[-2], img2.shape[-1])
    full[:, NCORES * BAND:] = _host_tail_products(im1, im2)
    # float64 integral image -> 15x15 box sums at the query points
    S = np.zeros((5, NROWS + 1, XP + 1), np.float64)
    np.cumsum(full, axis=1, dtype=np.float64, out=S[:, 1:, 1:])
    np.cumsum(S[:, 1:, 1:], axis=2, out=S[:, 1:, 1:])
    xs = points[:, 0].astype(np.int64)
    ys = points[:, 1].astype(np.int64)
    box = (S[:, ys + PATCH, xs + PATCH] - S[:, ys, xs + PATCH]
           - S[:, ys + PATCH, xs] + S[:, ys, xs])        # [5, N]
    a, h01, d, b0, b1 = box
    det = a * d - h01 * h01
    dx = (d * b0 - h01 * b1) / det
    dy = (a * b1 - h01 * b0) / det
    return np.stack([dx, dy], axis=-1).astype(np.float32)


def _run(img1, img2, points, trace=False):
    in_maps = build_core_inputs(img1, img2)
    nc = build_program()
    res = run_bass_kernel_spmd(nc, in_maps, list(range(NCORES)), trace=trace)
    pA = np.stack([np.asarray(res.results[c]["outA"]) for c in range(NCORES)])
    full = _solve_host(pA, img1, img2, np.asarray(points))
    return full, res


def kernel(img1, img2, points1):
    full, _ = _run(np.asarray(img1), np.asarray(img2), np.asarray(points1))
    return full


# revision 30
# speedup vs baseline: 1.6112x; 1.0002x over previous
"""Lucas-Kanade delta_p kernel for 8 trn2 NeuronCores.

Strategy (dense per-pixel product maps, no on-device gather):
Every per-point output derives from 15x15 box-sums of five per-pixel
product maps (Ix^2, IxIy, Iy^2, Ix*E, Iy*E with E = img2-img1).  Points
lie in [0,1000)^2 so only the top-left ~1016x1016 corner matters.  The
box-sums are evaluated on the host from an integral image, so the cores
produce DISJOINT row bands of the product maps (no halo): each of the 8
cores computes 126 sobel rows from a 128-row image slice:
 - full Sobel (vertical taps via banded lhsT, horizontal taps via
   shifted rhs views) as accumulating bf16 matmuls on the PE
 - the five per-pixel product maps on ACT (squares) / DVE / GpSimd,
   written as bf16 directly into the output staging tile
The host computes the last 6 product rows (1008..1013) directly in
numpy, builds a float64 integral image per map, and finishes with the
closed-form 2x2 solve at the 100k point locations.  No cross-core
communication, no gather.
"""

import numpy as np
import ml_dtypes

import concourse.bass as bass
import concourse.bacc as bacc
import concourse.mybir as mybir
from concourse.tile import TileContext
from concourse.bass_utils import run_bass_kernel_spmd

F32 = mybir.dt.float32
BF16 = mybir.dt.bfloat16

NCORES = 8
BAND = 126          # sobel/product rows per core (disjoint)
TA = 128            # image rows loaded per core
NROWS = 1014        # product rows needed globally (y+u <= 999+14)
CLD = 1040          # image columns loaded (shifted reads up to 1026)
CW = 1024           # working column width
XP = 1016           # product-map x columns that matter
PATCH = 15

AL = mybir.AluOpType
AF = mybir.ActivationFunctionType

_WO = {"sm": 0, "smn": 128, "df": 256, "df2": 384}


def _packed_weights():
    sm = (2.0, 4.0, 2.0)
    df = (2.0, 0.0, -2.0)
    smA = np.zeros((128, BAND), np.float32)
    dfA = np.zeros((128, BAND), np.float32)
    for m in range(BAND):
        for u in range(3):
            smA[m + u, m] = sm[u]
            dfA[m + u, m] = df[u]
    wp = np.zeros((128, 512), np.float32)
    for nm, blk in (("sm", smA), ("smn", -smA), ("df", dfA),
                    ("df2", 2.0 * dfA)):
        wp[:, _WO[nm]:_WO[nm] + BAND] = blk
    return wp.astype(ml_dtypes.bfloat16)


def build_core_inputs(img1, img2):
    im1 = np.asarray(img1).reshape(img1.shape[-2], img1.shape[-1])
    im2 = np.asarray(img2).reshape(img2.shape[-2], img2.shape[-1])
    wp = _packed_weights()
    in_maps = []
    for c in range(NCORES):
        r0 = c * BAND
        in_maps.append(dict(
            img1b=np.ascontiguousarray(
                im1[r0:r0 + TA, :CLD].astype(ml_dtypes.bfloat16)),
            img2b=np.ascontiguousarray(
                im2[r0:r0 + TA, :CLD].astype(ml_dtypes.bfloat16)),
            wp=wp))
    return in_maps


_prog_cache = {}


def build_program():
    if "p" in _prog_cache:
        return _prog_cache["p"]
    nc = bacc.Bacc(None, target_bir_lowering=False, debug=True)
    img1b = nc.declare_dram_parameter("img1b", [TA, CLD], BF16, isOutput=False)
    img2b = nc.declare_dram_parameter("img2b", [TA, CLD], BF16, isOutput=False)
    wp_d = nc.declare_dram_parameter("wp", [128, 512], BF16, isOutput=False)
    # per-partition free layout: [chunk(2), map(5), 512]
    outA = nc.declare_dram_parameter("outA", [BAND, 5120], BF16, isOutput=True)

    with TileContext(nc) as tc:
        with tc.tile_pool(name="cn", bufs=1) as cn, \
             tc.tile_pool(name="ps", bufs=8, space="PSUM") as ps:
            # ---- loads ---------------------------------------------------
            i1A = cn.tile([TA, CLD], BF16, tag="i1A")
            i2A = cn.tile([TA, CLD], BF16, tag="i2A")
            wp = cn.tile([128, 512], BF16, tag="wp")
            nc.sync.dma_start(out=i1A[:], in_=img1b[:])
            nc.scalar.dma_start(out=wp[:], in_=wp_d[:])
            nc.scalar.dma_start(out=i2A[0:64, :], in_=img2b[0:64, :])
            nc.sync.dma_start(out=i2A[64:TA, :], in_=img2b[64:TA, :])

            def W(name):
                return wp[:, _WO[name]:_WO[name] + BAND]

            # ---- persistent SBUF tiles ----------------------------------
            IyAs = cn.tile([BAND, CW], F32, tag="IyAs")
            EA = cn.tile([BAND, CW], F32, tag="EA")
            ot = cn.tile([BAND, 5120], BF16, tag="ot")

            CHUNKS = ((0, 384), (384, 384), (768, 256))
            OTOFF = (0, 1920, 3840)
            for ic, (c0, cw) in enumerate(CHUNKS):
                def sh(s):
                    return i1A[:, c0 + s:c0 + s + cw]
                o = slice(c0, c0 + cw)
                # Sobel: Ix = vsm[c] - vsm[c+2]; Iy = vdf[c]+2vdf[c+1]+vdf[c+2]
                Ix = ps.tile([BAND, cw], F32, tag="bank", name=f"Ix{ic}")
                nc.tensor.matmul(out=Ix[:], lhsT=W("sm"), rhs=sh(0),
                                 start=True, stop=False)
                nc.tensor.matmul(out=Ix[:], lhsT=W("smn"), rhs=sh(2),
                                 start=False, stop=True)
                Iy = ps.tile([BAND, cw], F32, tag="bank", name=f"Iy{ic}")
                nc.tensor.matmul(out=Iy[:], lhsT=W("df"), rhs=sh(0),
                                 start=True, stop=False)
                nc.tensor.matmul(out=Iy[:], lhsT=W("df2"), rhs=sh(1),
                                 start=False, stop=False)
                nc.tensor.matmul(out=Iy[:], lhsT=W("df"), rhs=sh(2),
                                 start=False, stop=True)

                nc.vector.tensor_tensor(out=EA[:, o], in0=i2A[0:BAND, o],
                                        in1=i1A[0:BAND, o], op=AL.subtract)
                nc.scalar.copy(out=IyAs[:, o], in_=Iy[:])

                # products straight into the bf16 staging tile
                def dst(ci):
                    base = OTOFF[ic] + ci * cw
                    return ot[:, base:base + cw]

                nc.scalar.activation(out=dst(0), in_=Ix[:], func=AF.Square)
                nc.scalar.activation(out=dst(2), in_=IyAs[:, o],
                                     func=AF.Square)
                nc.vector.tensor_tensor(out=dst(1), in0=Ix[:],
                                        in1=IyAs[:, o], op=AL.mult)
                nc.vector.tensor_tensor(out=dst(3), in0=Ix[:],
                                        in1=EA[:, o], op=AL.mult)
                nc.gpsimd.tensor_tensor(out=dst(4), in0=IyAs[:, o],
                                        in1=EA[:, o], op=AL.mult)

                oc = slice(OTOFF[ic], OTOFF[ic] + 5 * cw)
                if ic == 0:
                    nc.sync.dma_start(out=outA[:, oc], in_=ot[:, oc])
                elif ic == 1:
                    nc.scalar.dma_start(out=outA[:, oc], in_=ot[:, oc])
                else:
                    nc.gpsimd.dma_start(out=outA[0:100, oc], in_=ot[0:100, oc])
                    nc.sync.dma_start(out=outA[100:BAND, oc],
                                      in_=ot[100:BAND, oc])

    nc.compile()
    _prog_cache["p"] = nc
    return nc


def _host_tail_products(im1, im2):
    """Product-map rows 1008..1013 (not covered by the 8 cores), float64."""
    r0, r1 = NCORES * BAND, NROWS
    need = r1 - r0                         # 6 rows
    a = im1[r0:r1 + 2, :XP + 2].astype(np.float64)
    b = im2[r0:r1, :XP].astype(np.float64)
    sm = np.array([2.0, 4.0, 2.0])
    df = np.array([2.0, 0.0, -2.0])
    vs = sum(sm[u] * a[u:u + need] for u in range(3))
    vd = sum(df[u] * a[u:u + need] for u in range(3))
    ix = vs[:, 0:XP] - vs[:, 2:XP + 2]
    t = vd[:, 0:XP + 1] + vd[:, 1:XP + 2]
    iy = t[:, 0:XP] + t[:, 1:XP + 1]
    e = b - im1[r0:r1, :XP].astype(np.float64)
    return np.stack([ix * ix, ix * iy, iy * iy, ix * e, iy * e])


_CHUNKS = ((0, 384), (384, 384), (768, 256))
_OTOFF = (0, 1920, 3840)


def _solve_host(pA, img1, img2, points):
    # pA: [NCORES, BAND, 5120] bf16; per row: [5, cw] per chunk, concat
    pA = pA.astype(np.float32)
    maps = np.empty((NCORES, BAND, 5, CW), np.float32)
    for (c0, cw), off in zip(_CHUNKS, _OTOFF):
        blk = pA[:, :, off:off + 5 * cw].reshape(NCORES, BAND, 5, cw)
        maps[:, :, :, c0:c0 + cw] = blk
    full = np.empty((5, NROWS, XP), np.float32)
    full[:, :NCORES * BAND] = maps[:, :, :, :XP].transpose(2, 0, 1, 3).reshape(
        5, NCORES * BAND, XP)
    im1 = np.asarray(img1).reshape(img1.shape[-2], img1.shape[-1])
    im2 = np.asarray(img2).reshape(img2.shape[-2], img2.shape[-1])
    full[:, NCORES * BAND:] = _host_tail_products(im1, im2)
    # float64 integral image -> 15x15 box sums at the query points
    S = np.zeros((5, NROWS + 1, XP + 1), np.float64)
    np.cumsum(full, axis=1, dtype=np.float64, out=S[:, 1:, 1:])
    np.cumsum(S[:, 1:, 1:], axis=2, out=S[:, 1:, 1:])
    xs = points[:, 0].astype(np.int64)
    ys = points[:, 1].astype(np.int64)
    box = (S[:, ys + PATCH, xs + PATCH] - S[:, ys, xs + PATCH]
           - S[:, ys + PATCH, xs] + S[:, ys, xs])        # [5, N]
    a, h01, d, b0, b1 = box
    det = a * d - h01 * h01
    dx = (d * b0 - h01 * b1) / det
    dy = (a * b1 - h01 * b0) / det
    return np.stack([dx, dy], axis=-1).astype(np.float32)


def _run(img1, img2, points, trace=False):
    in_maps = build_core_inputs(img1, img2)
    nc = build_program()
    res = run_bass_kernel_spmd(nc, in_maps, list(range(NCORES)), trace=trace)
    pA = np.stack([np.asarray(res.results[c]["outA"]) for c in range(NCORES)])
    full = _solve_host(pA, img1, img2, np.asarray(points))
    return full, res


def kernel(img1, img2, points1):
    full, _ = _run(np.asarray(img1), np.asarray(img2), np.asarray(points1))
    return full


# revision 32
# speedup vs baseline: 1.6140x; 1.0017x over previous
"""Lucas-Kanade delta_p kernel for 8 trn2 NeuronCores.

Strategy (dense per-pixel product maps, no on-device gather):
Every per-point output derives from 15x15 box-sums of five per-pixel
product maps (Ix^2, IxIy, Iy^2, Ix*E, Iy*E with E = img2-img1).  Points
lie in [0,1000)^2 so only the top-left ~1016x1016 corner matters.  The
box-sums are evaluated on the host from an integral image, so the cores
produce DISJOINT row bands of the product maps (no halo): each of the 8
cores computes 126 sobel rows from a 128-row image slice:
 - full Sobel (vertical taps via banded lhsT, horizontal taps via
   shifted rhs views) as accumulating bf16 matmuls on the PE
 - the five per-pixel product maps on ACT (squares) / DVE / GpSimd,
   written as bf16 directly into the output staging tile
The host computes the last 6 product rows (1008..1013) directly in
numpy, builds a float64 integral image per map, and finishes with the
closed-form 2x2 solve at the 100k point locations.  No cross-core
communication, no gather.
"""

import numpy as np
import ml_dtypes

import concourse.bass as bass
import concourse.bacc as bacc
import concourse.mybir as mybir
from concourse.tile import TileContext
from concourse.bass_utils import run_bass_kernel_spmd

F32 = mybir.dt.float32
BF16 = mybir.dt.bfloat16

NCORES = 8
BAND = 126          # sobel/product rows per core (disjoint)
TA = 128            # image rows loaded per core
NROWS = 1014        # product rows needed globally (y+u <= 999+14)
CLD = 1040          # image columns loaded (shifted reads up to 1026)
CW = 1024           # working column width
XP = 1016           # product-map x columns that matter
PATCH = 15

AL = mybir.AluOpType
AF = mybir.ActivationFunctionType

_WO = {"sm": 0, "smn": 128, "df": 256, "df2": 384}


def _packed_weights():
    sm = (2.0, 4.0, 2.0)
    df = (2.0, 0.0, -2.0)
    smA = np.zeros((128, BAND), np.float32)
    dfA = np.zeros((128, BAND), np.float32)
    for m in range(BAND):
        for u in range(3):
            smA[m + u, m] = sm[u]
            dfA[m + u, m] = df[u]
    wp = np.zeros((128, 512), np.float32)
    for nm, blk in (("sm", smA), ("smn", -smA), ("df", dfA),
                    ("df2", 2.0 * dfA)):
        wp[:, _WO[nm]:_WO[nm] + BAND] = blk
    return wp.astype(ml_dtypes.bfloat16)


def build_core_inputs(img1, img2):
    im1 = np.asarray(img1).reshape(img1.shape[-2], img1.shape[-1])
    im2 = np.asarray(img2).reshape(img2.shape[-2], img2.shape[-1])
    wp = _packed_weights()
    in_maps = []
    for c in range(NCORES):
        r0 = c * BAND
        in_maps.append(dict(
            img1b=np.ascontiguousarray(
                im1[r0:r0 + TA, :CLD].astype(ml_dtypes.bfloat16)),
            img2b=np.ascontiguousarray(
                im2[r0:r0 + TA, :CLD].astype(ml_dtypes.bfloat16)),
            wp=wp))
    return in_maps


_prog_cache = {}


def build_program():
    if "p" in _prog_cache:
        return _prog_cache["p"]
    nc = bacc.Bacc(None, target_bir_lowering=False, debug=True)
    img1b = nc.declare_dram_parameter("img1b", [TA, CLD], BF16, isOutput=False)
    img2b = nc.declare_dram_parameter("img2b", [TA, CLD], BF16, isOutput=False)
    wp_d = nc.declare_dram_parameter("wp", [128, 512], BF16, isOutput=False)
    # per-partition free layout: [chunk(2), map(5), 512]
    outA = nc.declare_dram_parameter("outA", [BAND, 5120], BF16, isOutput=True)

    with TileContext(nc) as tc:
        with tc.tile_pool(name="cn", bufs=1) as cn, \
             tc.tile_pool(name="ps", bufs=8, space="PSUM") as ps:
            # ---- loads ---------------------------------------------------
            i1A = cn.tile([TA, CLD], BF16, tag="i1A")
            i2A = cn.tile([TA, CLD], BF16, tag="i2A")
            wp = cn.tile([128, 512], BF16, tag="wp")
            nc.sync.dma_start(out=i1A[:], in_=img1b[:])
            nc.scalar.dma_start(out=wp[:], in_=wp_d[:])
            nc.scalar.dma_start(out=i2A[0:64, :], in_=img2b[0:64, :])
            nc.sync.dma_start(out=i2A[64:TA, :], in_=img2b[64:TA, :])

            def W(name):
                return wp[:, _WO[name]:_WO[name] + BAND]

            # ---- persistent SBUF tiles ----------------------------------
            IyAs = cn.tile([BAND, CW], F32, tag="IyAs")
            EA = cn.tile([BAND, CW], F32, tag="EA")
            ot = cn.tile([BAND, 5120], BF16, tag="ot")

            CHUNKS = ((0, 384), (384, 384), (768, 256))
            OTOFF = (0, 1920, 3840)
            for ic, (c0, cw) in enumerate(CHUNKS):
                def sh(s):
                    return i1A[:, c0 + s:c0 + s + cw]
                o = slice(c0, c0 + cw)
                # Sobel: Ix = vsm[c] - vsm[c+2]; Iy = vdf[c]+2vdf[c+1]+vdf[c+2]
                Ix = ps.tile([BAND, cw], F32, tag="bank", name=f"Ix{ic}")
                nc.tensor.matmul(out=Ix[:], lhsT=W("sm"), rhs=sh(0),
                                 start=True, stop=False)
                nc.tensor.matmul(out=Ix[:], lhsT=W("smn"), rhs=sh(2),
                                 start=False, stop=True)
                Iy = ps.tile([BAND, cw], F32, tag="bank", name=f"Iy{ic}")
                nc.tensor.matmul(out=Iy[:], lhsT=W("df"), rhs=sh(0),
                                 start=True, stop=False)
                nc.tensor.matmul(out=Iy[:], lhsT=W("df2"), rhs=sh(1),
                                 start=False, stop=False)
                nc.tensor.matmul(out=Iy[:], lhsT=W("df"), rhs=sh(2),
                                 start=False, stop=True)

                nc.vector.tensor_tensor(out=EA[:, o], in0=i2A[0:BAND, o],
                                        in1=i1A[0:BAND, o], op=AL.subtract)
                nc.scalar.copy(out=IyAs[:, o], in_=Iy[:])

                # products straight into the bf16 staging tile
                def dst(ci):
                    base = OTOFF[ic] + ci * cw
                    return ot[:, base:base + cw]

                nc.scalar.activation(out=dst(0), in_=Ix[:], func=AF.Square)
                nc.scalar.activation(out=dst(2), in_=IyAs[:, o],
                                     func=AF.Square)
                nc.vector.tensor_tensor(out=dst(1), in0=Ix[:],
                                        in1=IyAs[:, o], op=AL.mult)
                nc.vector.tensor_tensor(out=dst(3), in0=Ix[:],
                                        in1=EA[:, o], op=AL.mult)
                nc.gpsimd.tensor_tensor(out=dst(4), in0=IyAs[:, o],
                                        in1=EA[:, o], op=AL.mult)

                oc = slice(OTOFF[ic], OTOFF[ic] + 5 * cw)
                if ic == 0:
                    nc.sync.dma_start(out=outA[:, oc], in_=ot[:, oc])
                elif ic == 1:
                    nc.scalar.dma_start(out=outA[:, oc], in_=ot[:, oc])
                else:
                    nc.gpsimd.dma_start(out=outA[0:100, oc], in_=ot[0:100, oc])
                    nc.sync.dma_start(out=outA[100:BAND, oc],
                                      in_=ot[100:BAND, oc])

    nc.compile()
    _prog_cache["p"] = nc
    return nc


def _host_tail_products(im1, im2):
    """Product-map rows 1008..1013 (not covered by the 8 cores), float64."""
    r0, r1 = NCORES * BAND, NROWS
    need = r1 - r0                         # 6 rows
    a = im1[r0:r1 + 2, :XP + 2].astype(np.float64)
    b = im2[r0:r1, :XP].astype(np.float64)
    sm = np.array([2.0, 4.0, 2.0])
    df = np.array([2.0, 0.0, -2.0])
    vs = sum(sm[u] * a[u:u + need] for u in range(3))
    vd = sum(df[u] * a[u:u + need] for u in range(3))
    ix = vs[:, 0:XP] - vs[:, 2:XP + 2]
    t = vd[:, 0:XP + 1] + vd[:, 1:XP + 2]
    iy = t[:, 0:XP] + t[:, 1:XP + 1]
    e = b - im1[r0:r1, :XP].astype(np.float64)
    return np.stack([ix * ix, ix * iy, iy * iy, ix * e, iy * e])


_CHUNKS = ((0, 384), (384, 384), (768, 256))
_OTOFF = (0, 1920, 3840)


def _solve_host(pA, img1, img2, points):
    # pA: [NCORES, BAND, 5120] bf16; per row: [5, cw] per chunk, concat
    pA = pA.astype(np.float32)
    maps = np.empty((NCORES, BAND, 5, CW), np.float32)
    for (c0, cw), off in zip(_CHUNKS, _OTOFF):
        blk = pA[:, :, off:off + 5 * cw].reshape(NCORES, BAND, 5, cw)
        maps[:, :, :, c0:c0 + cw] = blk
    full = np.empty((5, NROWS, XP), np.float32)
    full[:, :NCORES * BAND] = maps[:, :, :, :XP].transpose(2, 0, 1, 3).reshape(
        5, NCORES * BAND, XP)
    im1 = np.asarray(img1).reshape(img1.shape[-2], img1.shape[-1])
    im2 = np.asarray(img2).reshape(img2.shape[-2], img2.shape[-1])
    full[:, NCORES * BAND:] = _host_tail_products(im1, im2)
    # float64 integral image -> 15x15 box sums at the query points
    S = np.zeros((5, NROWS + 1, XP + 1), np.float64)
    np.cumsum(full, axis=1, dtype=np.float64, out=S[:, 1:, 1:])
    np.cumsum(S[:, 1:, 1:], axis=2, out=S[:, 1:, 1:])
    xs = points[:, 0].astype(np.int64)
    ys = points[:, 1].astype(np.int64)
    box = (S[:, ys + PATCH, xs + PATCH] - S[:, ys, xs + PATCH]
           - S[:, ys + PATCH, xs] + S[:, ys, xs])        # [5, N]
    a, h01, d, b0, b1 = box
    det = a * d - h01 * h01
    dx = (d * b0 - h01 * b1) / det
    dy = (a * b1 - h01 * b0) / det
    return np.stack([dx, dy], axis=-1).astype(np.float32)


def _run(img1, img2, points, trace=False):
    in_maps = build_core_inputs(img1, img2)
    nc = build_program()
    res = run_bass_kernel_spmd(nc, in_maps, list(range(NCORES)), trace=trace)
    pA = np.stack([np.asarray(res.results[c]["outA"]) for c in range(NCORES)])
    full = _solve_host(pA, img1, img2, np.asarray(points))
    return full, res


def kernel(img1, img2, points1):
    full, _ = _run(np.asarray(img1), np.asarray(img2), np.asarray(points1))
    return full
